# revision 8
# baseline (speedup 1.0000x reference)
"""Trainium2 Bass kernel for nn_Coembedding (dual-MLP cosine-similarity retrieval).

Computation (see reference):
    mp = relu(molecule @ Wm1.T + bm1) @ Wm2.T + bm2          [N, D]
    pp = relu(protein  @ Wp1.T + bp1) @ Wp2.T + bp2          [M, D]
    out = (pp/|pp| @ (mp/|mp|).T) / temperature              [M, N]

Distribution over 8 NeuronCores:
  - molecule rows (N) sharded 8x for the molecule MLP; normalized embeddings
    (feature-major [D, N/8]) AllGathered so every core holds all N molecule
    embeddings.
  - protein rows (M) sharded 8x; each core computes its own protein MLP shard
    and the [M/8, N] similarity tile.

All on-chip layouts are feature-major (K on partitions) so the two MLP layers
and the similarity GEMM chain without transposes.  The whole chain runs in
bf16 (weights, inputs, hidden, embeddings, output; fp32 PSUM accumulation):
simulated end-to-end rel-err 5.2e-3 vs the 2e-2 budget.  bf16 halves the
16MB/core weight stream that gated the MLP phase, halves LDWEIGHTS time
(109ns vs 215ns fp32r), and halves the 16MB output write.  fp8e4 DoubleRow
(2x matmul rate) was evaluated for the similarity GEMM and REJECTED: e4m3's
3 mantissa bits give 2.9e-2 rel-err (over budget), and 3-pass compensated
fp8 costs 1.5x bf16.

Schedule notes (trace-driven):
  - HWDGE queue bandwidth scales with DMA descriptor size (~150GB/s at 3KB
    per partition row vs ~340GB/s at 40KB).  Weight slabs are therefore
    host-packed in PAIRS of output chunks so each weight DMA moves 6-10KB
    per partition row, and the whole weight stream fits on the sync queue.
  - the scalar engine issues NO DMAs: a dma_start blocked on a buffer-reuse
    semaphore stalls the in-order engine and with it all PSUM-draining
    activations queued behind it (this cost 13us/run when weight loads
    alternated onto the scalar queue).
  - molT is split across the sync and gpsimd queues and the first weight
    pair is hoisted ahead of it, so the first matmul fires at ~14us.
  - norms use clamp(norm^2) -> sqrt -> broadcast -> reciprocal on 128-lane
    tiles; a [1, n] vector reciprocal runs on a single DVE lane (6.5us for
    n=1024, measured).
  - molecule normalization muls are split across vector and gpsimd and the
    collective-buffer sends are batched into 2 DMAs, shortening the
    AllGather doorbell path.
  - similarity output writes alternate between the gpsimd and sync queues
    (a single queue backlogs ~7us of writes at kernel end).
"""

import numpy as np
import ml_dtypes
from contextlib import ExitStack

BF16_NP = ml_dtypes.bfloat16

import concourse.bass as bass
import concourse.tile as tile
from concourse import bacc, mybir
from concourse.bass_utils import run_bass_kernel_spmd

F32 = mybir.dt.float32
F32R = mybir.dt.float32r
BF16 = mybir.dt.bfloat16
AF = mybir.ActivationFunctionType

N_CORES = 8
N, M, MOL, PROT, D = 4096, 8192, 768, 1280, 1024
NS = N // N_CORES            # 512 molecule rows per core
MS = M // N_CORES            # 1024 protein rows per core
KM, KP, KD = MOL // 128, PROT // 128, D // 128   # 6, 10, 8 contraction chunks
DC = D // 128                # 8 output-feature chunks
GC = DC // 2                 # 4 packed weight-pair groups
EPS = 1e-8

_CACHE: dict = {}


def _build():
    if "nc" in _CACHE:
        return _CACHE["nc"]

    nc = bacc.Bacc("TRN2", target_bir_lowering=False, debug=False,
                   num_devices=N_CORES)

    # All inputs pre-tiled host-side; every DMA below is partition-major
    # linear with large per-partition contiguous runs.
    molT = nc.dram_tensor("molT", [128, KM, NS], BF16, kind="ExternalInput").ap()
    protT = nc.dram_tensor("protT", [128, KP, MS], BF16, kind="ExternalInput").ap()
    wm1 = nc.dram_tensor("wm1", [GC, 128, 2 * KM * 128], BF16, kind="ExternalInput").ap()
    wm2 = nc.dram_tensor("wm2", [GC, 128, 2 * KD * 128], BF16, kind="ExternalInput").ap()
    wp1 = nc.dram_tensor("wp1", [GC, 128, 2 * KP * 128], BF16, kind="ExternalInput").ap()
    wp2 = nc.dram_tensor("wp2", [GC, 128, 2 * KD * 128], BF16, kind="ExternalInput").ap()
    bm1 = nc.dram_tensor("bm1", [128, DC], F32, kind="ExternalInput").ap()
    bm2 = nc.dram_tensor("bm2", [128, DC], F32, kind="ExternalInput").ap()
    bp1 = nc.dram_tensor("bp1", [128, DC], F32, kind="ExternalInput").ap()
    bp2 = nc.dram_tensor("bp2", [128, DC], F32, kind="ExternalInput").ap()
    invtemp = nc.dram_tensor("invtemp", [1, 1], F32, kind="ExternalInput").ap()
    ones_d = nc.dram_tensor("ones", [128, 128], F32R, kind="ExternalInput").ap()
    S = nc.dram_tensor("S", [N_CORES, DC, 128, NS], BF16, kind="ExternalOutput").ap()

    with tile.TileContext(nc) as tc, ExitStack() as ctx, \
            nc.allow_low_precision(reason="float32r tiles are bit-identical fp32"):
        dram = ctx.enter_context(tc.tile_pool(name="dram", bufs=1, space="DRAM"))
        send = dram.tile([128, DC, NS], BF16)            # Mn shard, partition-major
        recv = dram.tile([N_CORES, 128, DC, NS], BF16, addr_space="Shared")

        sb = ctx.enter_context(tc.tile_pool(name="sb", bufs=1))
        wstream = ctx.enter_context(tc.tile_pool(name="w", bufs=3))
        mn_pool = ctx.enter_context(tc.tile_pool(name="mn", bufs=2))
        st_pool = ctx.enter_context(tc.tile_pool(name="st", bufs=4))
        ps = ctx.enter_context(tc.tile_pool(name="ps", bufs=6, space="PSUM"))
        psn = ctx.enter_context(tc.tile_pool(name="psn", bufs=1, space="PSUM"))
        psb = ctx.enter_context(tc.tile_pool(name="psb", bufs=1, space="PSUM"))

        # ---- first molecule weight pair, hoisted ahead of everything on
        # the sync queue so the first matmul can fire as soon as possible ----
        wpair0_m1 = wstream.tile([128, 2, KM, 128], BF16, tag="wcol")
        w0v = wm1[0].rearrange("p (j k m) -> p j k m", j=2, k=KM)
        nc.sync.dma_start(out=wpair0_m1[:, 0:1], in_=w0v[:, 0:1])
        nc.sync.dma_start(out=wpair0_m1[:, 1:2], in_=w0v[:, 1:2])

        # molecule input, split over two queues / two tiles so the first
        # matmuls only gate on the first chunk
        molT_a = sb.tile([128, 2, NS], BF16, tag="molTa")
        nc.sync.dma_start(out=molT_a[:], in_=molT[:, 0:2, :])
        molT_b = sb.tile([128, KM - 2, NS], BF16, tag="molTb")
        nc.gpsimd.dma_start(out=molT_b[:], in_=molT[:, 2:, :])

        # ---- constants ----
        ones_col = sb.tile([128, 1], F32R, tag="ones_col")
        nc.gpsimd.dma_start(out=ones_col[:], in_=ones_d[:, 0:1])
        ones_row = sb.tile([1, 128], F32R, tag="ones_row")
        nc.gpsimd.dma_start(out=ones_row[:], in_=ones_d[0:1, :])
        invt = sb.tile([128, 1], F32, tag="invt")
        nc.gpsimd.dma_start(out=invt[:], in_=invtemp.to_broadcast([128, 1]))

        def load_bias(name, ap):
            t = sb.tile([128, DC], F32, tag=name)
            nc.gpsimd.dma_start(out=t[:], in_=ap[:])
            return t

        bm1_s, bm2_s = load_bias("bm1", bm1), load_bias("bm2", bm2)
        bp1_s, bp2_s = load_bias("bp1", bp1), load_bias("bp2", bp2)

        # warm the Sqrt activation table: the first Sqrt otherwise pays a
        # 1.3us ACT_TABLE_LOAD on the AllGather doorbell path
        sqrt_warm = sb.tile([1, 1], F32, tag="sqrt_warm")
        nc.scalar.activation(sqrt_warm[:], ones_col[0:1, 0:1], AF.Sqrt)

        # protein input rides the (otherwise idle until the sends) gpsimd
        # queue so sync is free for weight streaming.  It is issued as k-pair
        # DMAs (8KB descriptors): a single 40KB-descriptor DMA monopolizes
        # the per-core HBM read bandwidth (~344 of ~358GB/s) and starves the
        # weight stream; pairs hold it to ~150GB/s.  Two tiles so protein L1
        # can start before the tail chunks land.
        protT_a = sb.tile([128, 4, MS], BF16, tag="protTa")
        for kk in range(0, 4, 2):
            nc.gpsimd.dma_start(out=protT_a[:, kk:kk + 2, :],
                                in_=protT[:, kk:kk + 2, :])
        protT_b = sb.tile([128, KP - 4, MS], BF16, tag="protTb")
        for kk in range(4, KP, 2):
            nc.gpsimd.dma_start(out=protT_b[:, kk - 4:kk - 2, :],
                                in_=protT[:, kk:kk + 2, :])

        def mlp_layer(xs, w_dram, kchunks, ncols, bias_tile, relu, out_tile,
                      wpair0=None):
            """out[128, DC, ncols] = act(w.T @ x + b); all feature-major.
            xs is a list of (tile, local_k) per contraction chunk; weights
            stream as packed pairs of output chunks on the sync queue."""
            nhalves = ncols // 512
            for g in range(GC):
                if g == 0 and wpair0 is not None:
                    wpair = wpair0
                else:
                    wpair = wstream.tile([128, 2, kchunks, 128], BF16,
                                         tag="wcol")
                    nc.sync.dma_start(
                        out=wpair[:],
                        in_=w_dram[g].rearrange("p (j k m) -> p j k m",
                                                j=2, k=kchunks))
                for j in range(2):
                    h = 2 * g + j
                    for nh in range(nhalves):
                        pt = ps.tile([128, 512], F32, tag="mm")
                        for k in range(kchunks):
                            xt, lk = xs[k]
                            nc.tensor.matmul(
                                pt[:], wpair[:, j, k, :],
                                xt[:, lk, nh * 512:(nh + 1) * 512],
                                start=(k == 0), stop=(k == kchunks - 1),
                            )
                        nc.scalar.activation(
                            out_tile[:, h, nh * 512:(nh + 1) * 512], pt[:],
                            AF.Relu if relu else AF.Identity,
                            bias=bias_tile[:, h:h + 1],
                        )

        # ================= molecule MLP (N shard) =================
        xs_m1 = [(molT_a, k) for k in range(2)] + \
                [(molT_b, k) for k in range(KM - 2)]
        Hm = sb.tile([128, DC, NS], BF16, tag="hid")
        mlp_layer(xs_m1, wm1, KM, NS, bm1_s, True, Hm, wpair0=wpair0_m1)
        Mp = sb.tile([128, DC, NS], BF16, tag="emb")
        mlp_layer([(Hm, k) for k in range(KD)], wm2, KD, NS, bm2_s, False, Mp)

        # ---- normalize molecule embeddings, bf16, and send to the
        # collective buffer (2 batched DMAs) ----
        Mnb = sb.tile([128, DC, NS], BF16, tag="mnb")
        pn = psn.tile([1, NS], F32, tag="psn")
        for k in range(DC):
            sq = st_pool.tile([128, NS], F32R, tag="sq", bufs=4)
            eng = nc.vector if k % 2 == 0 else nc.gpsimd
            eng.tensor_mul(sq[:], Mp[:, k, :], Mp[:, k, :])
            nc.tensor.matmul(pn[:], ones_col[:], sq[:],
                             start=(k == 0), stop=(k == DC - 1))
        # clamp(norm^2) -> sqrt on the scalar engine (single-partition rows
        # are fine for tensor_scalar/activation but NOT vector.reciprocal,
        # which runs ~6.4ns/elem on one lane) -> broadcast the NORM to all
        # 128 partitions via an outer product -> reciprocal there.
        nsq = sb.tile([1, NS], F32, tag="normsq")
        nc.vector.tensor_scalar_max(nsq[:], pn[:], EPS * EPS)
        nrm = sb.tile([1, NS], F32R, tag="invn")
        nc.scalar.activation(nrm[:], nsq[:], AF.Sqrt)
        binv = sb.tile([128, NS], F32, tag="binv")
        pb = psb.tile([128, NS], F32, tag="psb")
        nc.tensor.matmul(pb[:], ones_row[:], nrm[:], start=True, stop=True)
        nc.vector.reciprocal_approx_fast(out=binv[:], in_=pb[:])
        # normalization muls split across vector (even k) and gpsimd (odd k)
        # so the send-side tail is ~half as long; sends batched per half.
        for k in range(DC):
            eng = nc.vector if k % 2 == 0 else nc.gpsimd
            eng.tensor_mul(Mnb[:, k, :], Mp[:, k, :], binv[:])
            if k == DC // 2 - 1:
                nc.gpsimd.dma_start(out=send[:, 0:DC // 2, :],
                                    in_=Mnb[:, 0:DC // 2, :])
        nc.gpsimd.dma_start(out=send[:, DC // 2:, :], in_=Mnb[:, DC // 2:, :])

        # ================= AllGather molecule embeddings =================
        nc.gpsimd.collective_compute(
            "AllGather",
            mybir.AluOpType.bypass,
            replica_groups=[list(range(N_CORES))],
            ins=[send[:]],
            outs=[recv[:]],
        )

        # ================= protein MLP (M shard) =================
        Hp = sb.tile([128, DC, MS], BF16, tag="hid")
        xs_p1 = [(protT_a, k) for k in range(4)] + \
                [(protT_b, k) for k in range(KP - 4)]
        mlp_layer(xs_p1, wp1, KP, MS, bp1_s, True, Hp)
        Pp = sb.tile([128, DC, MS], BF16, tag="emb")
        mlp_layer([(Hp, k) for k in range(KD)], wp2, KD, MS, bp2_s, False, Pp)
        # Pp is already bf16 out of the MLP; it feeds the similarity GEMM
        # directly (normalization folded into the S-tile eviction scale).
        # row-form clamp(|pp|^2) -> [1, MS] -> sqrt
        nsq_p = sb.tile([1, MS], F32, tag="normsq_p")
        for nh in range(MS // 512):
            pnp = psn.tile([1, 512], F32, tag="psn")
            for k in range(DC):
                sq = st_pool.tile([128, 512], F32R, tag="sq", bufs=4)
                nc.vector.tensor_mul(
                    sq[:], Pp[:, k, nh * 512:(nh + 1) * 512],
                    Pp[:, k, nh * 512:(nh + 1) * 512])
                nc.tensor.matmul(pnp[:], ones_col[:], sq[:],
                                 start=(k == 0), stop=(k == DC - 1))
            nc.vector.tensor_scalar_max(nsq_p[:, nh * 512:(nh + 1) * 512],
                                        pnp[:], EPS * EPS)
        nrm_p = sb.tile([1, MS], F32, tag="invn_p")
        nc.scalar.activation(nrm_p[:], nsq_p[:], AF.Sqrt)
        # transpose the NORM row [1, MS] -> column-form [128, DC] via 8
        # outer-product mms, then take the reciprocal in column form where
        # all 128 DVE lanes participate.
        ones_f32 = sb.tile([1, 1], F32, tag="ones_f32")
        nc.scalar.activation(ones_f32[:], ones_col[0:1, 0:1], AF.Copy)
        pcol = psb.tile([128, DC], F32, tag="psb")
        for j in range(DC):
            nc.tensor.matmul(pcol[:, j:j + 1], nrm_p[0:1, j * 128:(j + 1) * 128],
                             ones_f32[0:1, 0:1], start=(j == 0), stop=(j == DC - 1))
        scale_col = sb.tile([128, DC], F32, tag="scale_col")
        nc.vector.reciprocal_approx_fast(out=scale_col[:], in_=pcol[:])
        nc.vector.tensor_scalar_mul(scale_col[:], scale_col[:], invt[:, 0:1])

        # ================= similarity tiles =================
        for c in range(N_CORES):
            mnb = mn_pool.tile([128, DC, NS], BF16, tag="mn")
            if c == 0:
                # split load so the first matmul gates on one k-chunk
                nc.sync.dma_start(out=mnb[:, 0:1, :], in_=recv[c, :, 0:1, :])
                nc.sync.dma_start(out=mnb[:, 1:4, :], in_=recv[c, :, 1:4, :])
                nc.sync.dma_start(out=mnb[:, 4:, :], in_=recv[c, :, 4:, :])
            else:
                nc.sync.dma_start(out=mnb[:], in_=recv[c])
            for mi in range(MS // 128):
                pt = ps.tile([128, 512], F32, tag="mm")
                for k in range(KD):
                    nc.tensor.matmul(
                        pt[:], Pp[:, k, mi * 128:(mi + 1) * 128],
                        mnb[:, k, :],
                        start=(k == 0), stop=(k == KD - 1),
                    )
                stile = st_pool.tile([128, NS], BF16, tag="stile", bufs=3)
                nc.scalar.activation(stile[:], pt[:], AF.Copy,
                                     scale=scale_col[:, mi:mi + 1])
                (nc.gpsimd if mi % 2 == 0 else nc.sync).dma_start(
                    out=S[c, mi], in_=stile[:])

    nc.compile()
    _CACHE["nc"] = nc
    return nc


def _tile_w(W):
    """W [D, K] (fp32) -> [GC, 128, 2*K]: packed pairs of output chunks.
    Element (g, p, (j*kc + k)*128 + m) = W[(2g+j)*128+m, k*128+p]: each
    pair-slab is a linear partition-major load with 2*K*4B per row."""
    Dout, K = W.shape
    kc = K // 128
    t = W.reshape(GC, 2, 128, kc, 128)          # [g, j, m(out-row), k, p]
    t = t.transpose(0, 4, 1, 3, 2)              # [g, p, j, k, m]
    return np.ascontiguousarray(t.reshape(GC, 128, 2 * kc * 128).astype(BF16_NP))


def _tile_x(Xshard):
    """X [rows, K] -> [128, KC, rows] feature-major partition-tiled."""
    rows, K = Xshard.shape
    kc = K // 128
    t = Xshard.reshape(rows, kc, 128).transpose(2, 1, 0)    # [p, k, rows]
    return np.ascontiguousarray(t.astype(BF16_NP))


def kernel(molecule, protein, Wm1, bm1, Wm2, bm2, Wp1, bp1, Wp2, bp2,
           temperature):
    nc = _build()

    molecule = np.asarray(molecule, np.float32)
    protein = np.asarray(protein, np.float32)
    wm1 = _tile_w(np.asarray(Wm1, np.float32))
    wm2 = _tile_w(np.asarray(Wm2, np.float32))
    wp1 = _tile_w(np.asarray(Wp1, np.float32))
    wp2 = _tile_w(np.asarray(Wp2, np.float32))

    def tile_b(b):
        return np.ascontiguousarray(np.asarray(b, np.float32).reshape(DC, 128).T)

    bm1_np, bm2_np = tile_b(bm1), tile_b(bm2)
    bp1_np, bp2_np = tile_b(bp1), tile_b(bp2)
    invt = (1.0 / np.asarray(temperature, np.float32)).reshape(1, 1)
    ones_np = np.ones((128, 128), np.float32)

    in_maps = []
    for c in range(N_CORES):
        in_maps.append({
            "molT": _tile_x(molecule[c * NS:(c + 1) * NS]),
            "protT": _tile_x(protein[c * MS:(c + 1) * MS]),
            "wm1": wm1, "wm2": wm2, "wp1": wp1, "wp2": wp2,
            "bm1": bm1_np, "bm2": bm2_np, "bp1": bp1_np, "bp2": bp2_np,
            "invtemp": invt, "ones": ones_np,
        })

    _CACHE["in_maps"] = in_maps
    res = run_bass_kernel_spmd(nc, in_maps, list(range(N_CORES)))
    out = np.empty((M, N), np.float32)
    for c in range(N_CORES):
        # S block layout [c2, mi, 128, 512] -> rows mi*128+i, cols c2*512+j
        blk = res.results[c]["S"].astype(np.float32)   # [8, 8, 128, 512]
        out[c * MS:(c + 1) * MS] = blk.transpose(1, 2, 0, 3).reshape(MS, N)
    return out



# revision 9
# speedup vs baseline: 1.0130x; 1.0130x over previous
"""Trainium2 Bass kernel for nn_Coembedding (dual-MLP cosine-similarity retrieval).

Computation (see reference):
    mp = relu(molecule @ Wm1.T + bm1) @ Wm2.T + bm2          [N, D]
    pp = relu(protein  @ Wp1.T + bp1) @ Wp2.T + bp2          [M, D]
    out = (pp/|pp| @ (mp/|mp|).T) / temperature              [M, N]

Distribution over 8 NeuronCores:
  - molecule rows (N) sharded 8x for the molecule MLP; normalized embeddings
    (feature-major [D, N/8]) AllGathered so every core holds all N molecule
    embeddings.
  - protein rows (M) sharded 8x; each core computes its own protein MLP shard
    and the [M/8, N] similarity tile.

All on-chip layouts are feature-major (K on partitions) so the two MLP layers
and the similarity GEMM chain without transposes.  The whole chain runs in
bf16 (weights, inputs, hidden, embeddings, output; fp32 PSUM accumulation):
simulated end-to-end rel-err 5.2e-3 vs the 2e-2 budget.  bf16 halves the
16MB/core weight stream that gated the MLP phase, halves LDWEIGHTS time
(109ns vs 215ns fp32r), and halves the 16MB output write.  fp8e4 DoubleRow
(2x matmul rate) was evaluated for the similarity GEMM and REJECTED: e4m3's
3 mantissa bits give 2.9e-2 rel-err (over budget), and 3-pass compensated
fp8 costs 1.5x bf16.

Schedule notes (trace-driven):
  - HWDGE queue bandwidth scales with DMA descriptor size (~150GB/s at 3KB
    per partition row vs ~340GB/s at 40KB).  Weight slabs are therefore
    host-packed in PAIRS of output chunks so each weight DMA moves 6-10KB
    per partition row, and the whole weight stream fits on the sync queue.
  - the scalar engine issues NO DMAs: a dma_start blocked on a buffer-reuse
    semaphore stalls the in-order engine and with it all PSUM-draining
    activations queued behind it (this cost 13us/run when weight loads
    alternated onto the scalar queue).
  - molT is split across the sync and gpsimd queues and the first weight
    pair is hoisted ahead of it, so the first matmul fires at ~14us.
  - norms use clamp(norm^2) -> sqrt -> broadcast -> reciprocal on 128-lane
    tiles; a [1, n] vector reciprocal runs on a single DVE lane (6.5us for
    n=1024, measured).
  - molecule normalization muls are split across vector and gpsimd and the
    collective-buffer sends are batched into 2 DMAs, shortening the
    AllGather doorbell path.
  - similarity output writes alternate between the gpsimd and sync queues
    (a single queue backlogs ~7us of writes at kernel end).
"""

import numpy as np
import ml_dtypes
from contextlib import ExitStack

BF16_NP = ml_dtypes.bfloat16

import concourse.bass as bass
import concourse.tile as tile
from concourse import bacc, mybir
from concourse.bass_utils import run_bass_kernel_spmd

F32 = mybir.dt.float32
F32R = mybir.dt.float32r
BF16 = mybir.dt.bfloat16
AF = mybir.ActivationFunctionType

N_CORES = 8
N, M, MOL, PROT, D = 4096, 8192, 768, 1280, 1024
NS = N // N_CORES            # 512 molecule rows per core
MS = M // N_CORES            # 1024 protein rows per core
KM, KP, KD = MOL // 128, PROT // 128, D // 128   # 6, 10, 8 contraction chunks
DC = D // 128                # 8 output-feature chunks
GC = DC // 2                 # 4 packed weight-pair groups
EPS = 1e-8

_CACHE: dict = {}


def _build():
    if "nc" in _CACHE:
        return _CACHE["nc"]

    nc = bacc.Bacc("TRN2", target_bir_lowering=False, debug=False,
                   num_devices=N_CORES)

    # All inputs pre-tiled host-side; every DMA below is partition-major
    # linear with large per-partition contiguous runs.
    molT = nc.dram_tensor("molT", [128, KM, NS], BF16, kind="ExternalInput").ap()
    protT = nc.dram_tensor("protT", [128, KP, MS], BF16, kind="ExternalInput").ap()
    wm1 = nc.dram_tensor("wm1", [GC, 128, 2 * KM * 128], BF16, kind="ExternalInput").ap()
    wm2 = nc.dram_tensor("wm2", [GC, 128, 2 * KD * 128], BF16, kind="ExternalInput").ap()
    wp1 = nc.dram_tensor("wp1", [GC, 128, 2 * KP * 128], BF16, kind="ExternalInput").ap()
    wp2 = nc.dram_tensor("wp2", [GC, 128, 2 * KD * 128], BF16, kind="ExternalInput").ap()
    bm1 = nc.dram_tensor("bm1", [128, DC], F32, kind="ExternalInput").ap()
    bm2 = nc.dram_tensor("bm2", [128, DC], F32, kind="ExternalInput").ap()
    bp1 = nc.dram_tensor("bp1", [128, DC], F32, kind="ExternalInput").ap()
    bp2 = nc.dram_tensor("bp2", [128, DC], F32, kind="ExternalInput").ap()
    invtemp = nc.dram_tensor("invtemp", [1, 1], F32, kind="ExternalInput").ap()
    ones_d = nc.dram_tensor("ones", [128, 128], F32R, kind="ExternalInput").ap()
    S = nc.dram_tensor("S", [N_CORES, DC, 128, NS], BF16, kind="ExternalOutput").ap()

    with tile.TileContext(nc) as tc, ExitStack() as ctx, \
            nc.allow_low_precision(reason="float32r tiles are bit-identical fp32"):
        dram = ctx.enter_context(tc.tile_pool(name="dram", bufs=1, space="DRAM"))
        send = dram.tile([128, DC, NS], BF16)            # Mn shard, partition-major
        recv = dram.tile([N_CORES, 128, DC, NS], BF16, addr_space="Shared")

        sb = ctx.enter_context(tc.tile_pool(name="sb", bufs=1))
        wstream = ctx.enter_context(tc.tile_pool(name="w", bufs=4))
        mn_pool = ctx.enter_context(tc.tile_pool(name="mn", bufs=2))
        st_pool = ctx.enter_context(tc.tile_pool(name="st", bufs=4))
        ps = ctx.enter_context(tc.tile_pool(name="ps", bufs=6, space="PSUM"))
        psn = ctx.enter_context(tc.tile_pool(name="psn", bufs=1, space="PSUM"))
        psb = ctx.enter_context(tc.tile_pool(name="psb", bufs=1, space="PSUM"))

        # ---- first molecule weight pair, hoisted ahead of everything on
        # the sync queue so the first matmul can fire as soon as possible ----
        wpair0_m1 = wstream.tile([128, 2, KM, 128], BF16, tag="wcol")
        w0v = wm1[0].rearrange("p (j k m) -> p j k m", j=2, k=KM)
        nc.sync.dma_start(out=wpair0_m1[:, 0:1], in_=w0v[:, 0:1])
        nc.sync.dma_start(out=wpair0_m1[:, 1:2], in_=w0v[:, 1:2])

        # molecule input, split over two queues / two tiles so the first
        # matmuls only gate on the first chunk
        molT_a = sb.tile([128, 2, NS], BF16, tag="molTa")
        nc.sync.dma_start(out=molT_a[:], in_=molT[:, 0:2, :])
        molT_b = sb.tile([128, KM - 2, NS], BF16, tag="molTb")
        nc.gpsimd.dma_start(out=molT_b[:], in_=molT[:, 2:, :])

        # ---- constants ----
        ones_col = sb.tile([128, 1], F32R, tag="ones_col")
        nc.gpsimd.dma_start(out=ones_col[:], in_=ones_d[:, 0:1])
        ones_row = sb.tile([1, 128], F32R, tag="ones_row")
        nc.gpsimd.dma_start(out=ones_row[:], in_=ones_d[0:1, :])
        invt = sb.tile([128, 1], F32, tag="invt")
        nc.gpsimd.dma_start(out=invt[:], in_=invtemp.to_broadcast([128, 1]))

        def load_bias(name, ap):
            t = sb.tile([128, DC], F32, tag=name)
            nc.gpsimd.dma_start(out=t[:], in_=ap[:])
            return t

        bm1_s, bm2_s = load_bias("bm1", bm1), load_bias("bm2", bm2)
        bp1_s, bp2_s = load_bias("bp1", bp1), load_bias("bp2", bp2)

        # warm the Sqrt activation table: the first Sqrt otherwise pays a
        # 1.3us ACT_TABLE_LOAD on the AllGather doorbell path
        sqrt_warm = sb.tile([1, 1], F32, tag="sqrt_warm")
        nc.scalar.activation(sqrt_warm[:], ones_col[0:1, 0:1], AF.Sqrt)

        # protein input rides the (otherwise idle until the sends) gpsimd
        # queue so sync is free for weight streaming.  It is issued as k-pair
        # DMAs (8KB descriptors): a single 40KB-descriptor DMA monopolizes
        # the per-core HBM read bandwidth (~344 of ~358GB/s) and starves the
        # weight stream; pairs hold it to ~150GB/s.  Two tiles so protein L1
        # can start before the tail chunks land.
        protT_a = sb.tile([128, 4, MS], BF16, tag="protTa")
        for kk in range(0, 4, 2):
            nc.gpsimd.dma_start(out=protT_a[:, kk:kk + 2, :],
                                in_=protT[:, kk:kk + 2, :])
        protT_b = sb.tile([128, KP - 4, MS], BF16, tag="protTb")
        for kk in range(4, KP, 2):
            nc.gpsimd.dma_start(out=protT_b[:, kk - 4:kk - 2, :],
                                in_=protT[:, kk:kk + 2, :])

        def mlp_layer(xs, w_dram, kchunks, ncols, bias_tile, relu, out_tile,
                      wpair0=None):
            """out[128, DC, ncols] = act(w.T @ x + b); all feature-major.
            xs is a list of (tile, local_k) per contraction chunk; weights
            stream as packed pairs of output chunks on the sync queue."""
            nhalves = ncols // 512
            for g in range(GC):
                if g == 0 and wpair0 is not None:
                    wpair = wpair0
                else:
                    wpair = wstream.tile([128, 2, kchunks, 128], BF16,
                                         tag="wcol")
                    nc.sync.dma_start(
                        out=wpair[:],
                        in_=w_dram[g].rearrange("p (j k m) -> p j k m",
                                                j=2, k=kchunks))
                for j in range(2):
                    h = 2 * g + j
                    for nh in range(nhalves):
                        pt = ps.tile([128, 512], F32, tag="mm")
                        for k in range(kchunks):
                            xt, lk = xs[k]
                            nc.tensor.matmul(
                                pt[:], wpair[:, j, k, :],
                                xt[:, lk, nh * 512:(nh + 1) * 512],
                                start=(k == 0), stop=(k == kchunks - 1),
                            )
                        nc.scalar.activation(
                            out_tile[:, h, nh * 512:(nh + 1) * 512], pt[:],
                            AF.Relu if relu else AF.Identity,
                            bias=bias_tile[:, h:h + 1],
                        )

        # ================= molecule MLP (N shard) =================
        xs_m1 = [(molT_a, k) for k in range(2)] + \
                [(molT_b, k) for k in range(KM - 2)]
        Hm = sb.tile([128, DC, NS], BF16, tag="hid")
        mlp_layer(xs_m1, wm1, KM, NS, bm1_s, True, Hm, wpair0=wpair0_m1)
        Mp = sb.tile([128, DC, NS], BF16, tag="emb")
        mlp_layer([(Hm, k) for k in range(KD)], wm2, KD, NS, bm2_s, False, Mp)

        # ---- normalize molecule embeddings, bf16, and send to the
        # collective buffer (2 batched DMAs) ----
        Mnb = sb.tile([128, DC, NS], BF16, tag="mnb")
        pn = psn.tile([1, NS], F32, tag="psn")
        for k in range(DC):
            sq = st_pool.tile([128, NS], F32R, tag="sq", bufs=4)
            eng = nc.vector if k % 2 == 0 else nc.gpsimd
            eng.tensor_mul(sq[:], Mp[:, k, :], Mp[:, k, :])
            nc.tensor.matmul(pn[:], ones_col[:], sq[:],
                             start=(k == 0), stop=(k == DC - 1))
        # clamp(norm^2) -> sqrt on the scalar engine (single-partition rows
        # are fine for tensor_scalar/activation but NOT vector.reciprocal,
        # which runs ~6.4ns/elem on one lane) -> broadcast the NORM to all
        # 128 partitions via an outer product -> reciprocal there.
        nsq = sb.tile([1, NS], F32, tag="normsq")
        nc.vector.tensor_scalar_max(nsq[:], pn[:], EPS * EPS)
        nrm = sb.tile([1, NS], F32R, tag="invn")
        nc.scalar.activation(nrm[:], nsq[:], AF.Sqrt)
        binv = sb.tile([128, NS], F32, tag="binv")
        pb = psb.tile([128, NS], F32, tag="psb")
        nc.tensor.matmul(pb[:], ones_row[:], nrm[:], start=True, stop=True)
        nc.vector.reciprocal_approx_fast(out=binv[:], in_=pb[:])
        # normalization muls split across vector (even k) and gpsimd (odd k)
        # so the send-side tail is ~half as long; sends batched per half.
        for k in range(DC):
            eng = nc.vector if k % 2 == 0 else nc.gpsimd
            eng.tensor_mul(Mnb[:, k, :], Mp[:, k, :], binv[:])
            if k == DC // 2 - 1:
                nc.gpsimd.dma_start(out=send[:, 0:DC // 2, :],
                                    in_=Mnb[:, 0:DC // 2, :])
        nc.gpsimd.dma_start(out=send[:, DC // 2:, :], in_=Mnb[:, DC // 2:, :])

        # ================= AllGather molecule embeddings =================
        nc.gpsimd.collective_compute(
            "AllGather",
            mybir.AluOpType.bypass,
            replica_groups=[list(range(N_CORES))],
            ins=[send[:]],
            outs=[recv[:]],
        )

        # ================= protein MLP (M shard) =================
        Hp = sb.tile([128, DC, MS], BF16, tag="hid")
        xs_p1 = [(protT_a, k) for k in range(4)] + \
                [(protT_b, k) for k in range(KP - 4)]
        mlp_layer(xs_p1, wp1, KP, MS, bp1_s, True, Hp)
        Pp = sb.tile([128, DC, MS], BF16, tag="emb")
        mlp_layer([(Hp, k) for k in range(KD)], wp2, KD, MS, bp2_s, False, Pp)
        # Pp is already bf16 out of the MLP; it feeds the similarity GEMM
        # directly (normalization folded into the S-tile eviction scale).
        # row-form clamp(|pp|^2) -> [1, MS] -> sqrt
        nsq_p = sb.tile([1, MS], F32, tag="normsq_p")
        for nh in range(MS // 512):
            pnp = psn.tile([1, 512], F32, tag="psn")
            for k in range(DC):
                sq = st_pool.tile([128, 512], F32R, tag="sq", bufs=4)
                nc.vector.tensor_mul(
                    sq[:], Pp[:, k, nh * 512:(nh + 1) * 512],
                    Pp[:, k, nh * 512:(nh + 1) * 512])
                nc.tensor.matmul(pnp[:], ones_col[:], sq[:],
                                 start=(k == 0), stop=(k == DC - 1))
            nc.vector.tensor_scalar_max(nsq_p[:, nh * 512:(nh + 1) * 512],
                                        pnp[:], EPS * EPS)
        nrm_p = sb.tile([1, MS], F32, tag="invn_p")
        nc.scalar.activation(nrm_p[:], nsq_p[:], AF.Sqrt)
        # transpose the NORM row [1, MS] -> column-form [128, DC] via 8
        # outer-product mms, then take the reciprocal in column form where
        # all 128 DVE lanes participate.
        ones_f32 = sb.tile([1, 1], F32, tag="ones_f32")
        nc.scalar.activation(ones_f32[:], ones_col[0:1, 0:1], AF.Copy)
        pcol = psb.tile([128, DC], F32, tag="psb")
        for j in range(DC):
            nc.tensor.matmul(pcol[:, j:j + 1], nrm_p[0:1, j * 128:(j + 1) * 128],
                             ones_f32[0:1, 0:1], start=(j == 0), stop=(j == DC - 1))
        scale_col = sb.tile([128, DC], F32, tag="scale_col")
        nc.vector.reciprocal_approx_fast(out=scale_col[:], in_=pcol[:])
        nc.vector.tensor_scalar_mul(scale_col[:], scale_col[:], invt[:, 0:1])

        # ================= similarity tiles =================
        for c in range(N_CORES):
            mnb = mn_pool.tile([128, DC, NS], BF16, tag="mn")
            if c == 0:
                # split load so the first matmul gates on one k-chunk
                nc.sync.dma_start(out=mnb[:, 0:1, :], in_=recv[c, :, 0:1, :])
                nc.sync.dma_start(out=mnb[:, 1:4, :], in_=recv[c, :, 1:4, :])
                nc.sync.dma_start(out=mnb[:, 4:, :], in_=recv[c, :, 4:, :])
            else:
                nc.sync.dma_start(out=mnb[:], in_=recv[c])
            for mi in range(MS // 128):
                pt = ps.tile([128, 512], F32, tag="mm")
                for k in range(KD):
                    nc.tensor.matmul(
                        pt[:], Pp[:, k, mi * 128:(mi + 1) * 128],
                        mnb[:, k, :],
                        start=(k == 0), stop=(k == KD - 1),
                    )
                stile = st_pool.tile([128, NS], BF16, tag="stile", bufs=3)
                nc.scalar.activation(stile[:], pt[:], AF.Copy,
                                     scale=scale_col[:, mi:mi + 1])
                nc.gpsimd.dma_start(out=S[c, mi], in_=stile[:])

    nc.compile()
    _CACHE["nc"] = nc
    return nc


def _tile_w(W):
    """W [D, K] (fp32) -> [GC, 128, 2*K]: packed pairs of output chunks.
    Element (g, p, (j*kc + k)*128 + m) = W[(2g+j)*128+m, k*128+p]: each
    pair-slab is a linear partition-major load with 2*K*4B per row."""
    Dout, K = W.shape
    kc = K // 128
    t = W.reshape(GC, 2, 128, kc, 128)          # [g, j, m(out-row), k, p]
    t = t.transpose(0, 4, 1, 3, 2)              # [g, p, j, k, m]
    return np.ascontiguousarray(t.reshape(GC, 128, 2 * kc * 128).astype(BF16_NP))


def _tile_x(Xshard):
    """X [rows, K] -> [128, KC, rows] feature-major partition-tiled."""
    rows, K = Xshard.shape
    kc = K // 128
    t = Xshard.reshape(rows, kc, 128).transpose(2, 1, 0)    # [p, k, rows]
    return np.ascontiguousarray(t.astype(BF16_NP))


def kernel(molecule, protein, Wm1, bm1, Wm2, bm2, Wp1, bp1, Wp2, bp2,
           temperature):
    nc = _build()

    molecule = np.asarray(molecule, np.float32)
    protein = np.asarray(protein, np.float32)
    wm1 = _tile_w(np.asarray(Wm1, np.float32))
    wm2 = _tile_w(np.asarray(Wm2, np.float32))
    wp1 = _tile_w(np.asarray(Wp1, np.float32))
    wp2 = _tile_w(np.asarray(Wp2, np.float32))

    def tile_b(b):
        return np.ascontiguousarray(np.asarray(b, np.float32).reshape(DC, 128).T)

    bm1_np, bm2_np = tile_b(bm1), tile_b(bm2)
    bp1_np, bp2_np = tile_b(bp1), tile_b(bp2)
    invt = (1.0 / np.asarray(temperature, np.float32)).reshape(1, 1)
    ones_np = np.ones((128, 128), np.float32)

    in_maps = []
    for c in range(N_CORES):
        in_maps.append({
            "molT": _tile_x(molecule[c * NS:(c + 1) * NS]),
            "protT": _tile_x(protein[c * MS:(c + 1) * MS]),
            "wm1": wm1, "wm2": wm2, "wp1": wp1, "wp2": wp2,
            "bm1": bm1_np, "bm2": bm2_np, "bp1": bp1_np, "bp2": bp2_np,
            "invtemp": invt, "ones": ones_np,
        })

    _CACHE["in_maps"] = in_maps
    res = run_bass_kernel_spmd(nc, in_maps, list(range(N_CORES)))
    out = np.empty((M, N), np.float32)
    for c in range(N_CORES):
        # S block layout [c2, mi, 128, 512] -> rows mi*128+i, cols c2*512+j
        blk = res.results[c]["S"].astype(np.float32)   # [8, 8, 128, 512]
        out[c * MS:(c + 1) * MS] = blk.transpose(1, 2, 0, 3).reshape(MS, N)
    return out

